# revision 8
# baseline (speedup 1.0000x reference)
"""Trainium2 Bass kernel for the DVQ (decomposed vector-quantization) module.

Problem shapes (hardcoded): B=4096, D=512, K=1024, N=8, TEMP=0.5.

Sharding: one decomposition slice n per NeuronCore (8 cores = N=8).
  - replicated : inputs (transposed on host to [D, B])
  - sharded    : W_proj[:, n*D:(n+1)*D], b_proj[n*D:(n+1)*D], codebooks[n],
                 gumbel noise g[:, n, :]
  - gathered   : quantized (stack over n), encoding_indices (stack), loss
                 (mean of per-(b,n) KL sums; the "all-reduce" of the hint).

Math notes: with s[b,k] = 2*x.e - |e|^2, both argmin(distance) and
log_softmax(-distance) are invariant to the per-(b,n) shift |x|^2, so the
kernel never computes x2.  logits = s - log(sum_k exp(s));
KLsum(b) = sum_k p*s / Z - log Z + log K with p = exp(s), Z = sum p.
Gumbel-softmax: sample_probs = exp(2*(s+g)) / Zq (temp 0.5), and
quantized = (1/Zq) * sum_k exp(2(s+g))[k] * cb[k, :].
Value ranges keep exp() in f32 range without max-subtraction.
"""

import numpy as np

B, D, K, N = 4096, 512, 1024, 8
NCORES = 8
BT = B // 128          # 32 b-tiles of 128 rows
NCHUNK = 8             # b processed in 8 chunks of 512 rows
LOGK = float(np.log(K))

_cache = {}


def _gumbel() -> np.ndarray:
    """The reference's gumbel noise: jax.random.gumbel(key(1), (B, N, K))."""
    if "g" not in _cache:
        import jax

        with jax.default_device(jax.devices("cpu")[0]):
            g = jax.random.gumbel(
                jax.random.key(1), (B, N, K), dtype=np.float32
            )
            _cache["g"] = np.asarray(g)
    return _cache["g"]


def _build():
    """Build + compile the per-core Bass program (identical on all cores)."""
    if "nc" in _cache:
        return _cache["nc"]

    import concourse.bacc as bacc
    import concourse.mybir as mybir
    import concourse.tile as tile

    DT = mybir.dt.float32
    FR = mybir.dt.float32r
    AF = mybir.ActivationFunctionType
    OP = mybir.AluOpType

    nc = bacc.Bacc("TRN2", target_bir_lowering=False, debug=False,
                   num_devices=NCORES)

    inT_h = nc.dram_tensor("inT", [D, B], DT, kind="ExternalInput").ap()
    w_h = nc.dram_tensor("w", [D, D], DT, kind="ExternalInput").ap()
    bias_h = nc.dram_tensor("bias", [128, 4], DT, kind="ExternalInput").ap()
    cbT_h = nc.dram_tensor("cbT", [D, K], DT, kind="ExternalInput").ap()
    cb_h = nc.dram_tensor("cb", [K, D], FR, kind="ExternalInput").ap()
    g_h = nc.dram_tensor("g", [B, K], DT, kind="ExternalInput").ap()
    id_h = nc.dram_tensor("ident", [128, 128], FR, kind="ExternalInput").ap()

    quant_h = nc.dram_tensor("quant", [B, D], DT, kind="ExternalOutput").ap()
    idx_h = nc.dram_tensor("idx", [128, BT * 8], mybir.dt.uint32,
                           kind="ExternalOutput").ap()
    kl_h = nc.dram_tensor("kl", [128, BT], DT, kind="ExternalOutput").ap()

    with tile.TileContext(nc) as tc:
        with (
            tc.tile_pool(name="const", bufs=1) as cp,
            tc.tile_pool(name="stream", bufs=2) as sp,
            tc.tile_pool(name="small", bufs=1) as mp,
            tc.tile_pool(name="psum", bufs=4, space="PSUM") as pp,
        ):
            # ---- resident constants ----
            w_sb = cp.tile([128, 4, D], DT, tag="w")
            nc.sync.dma_start(w_sb[:], w_h.rearrange("(t p) j -> p t j", p=128))
            cbT_sb = cp.tile([128, 4, K], DT, tag="cbT")
            nc.sync.dma_start(cbT_sb[:], cbT_h.rearrange("(t p) k -> p t k", p=128))
            cb_sb = cp.tile([128, 8, D], FR, tag="cb")
            nc.sync.dma_start(cb_sb[:], cb_h.rearrange("(t p) d -> p t d", p=128))
            bias_sb = cp.tile([128, 4], DT, tag="bias")
            nc.sync.dma_start(bias_sb[:], bias_h[:])
            id_sb = cp.tile([128, 128], FR, tag="ident")
            nc.sync.dma_start(id_sb[:], id_h[:])

            ones_row = cp.tile([1, 128], DT, tag="ones_row")
            nc.gpsimd.memset(ones_row[:], 1.0)
            negones = cp.tile([128, 1], DT, tag="negones")
            nc.gpsimd.memset(negones[:], -1.0)

            # ---- e2row[0, k] = -sum_d cb[k, d]^2, in free-dim layout ----
            cbsq = cp.tile([128, 4, K], DT, tag="cbsq")
            for dt in range(4):
                nc.scalar.activation(cbsq[:, dt, :], cbT_sb[:, dt, :], AF.Square)
            e2ps = pp.tile([128, K], DT, tag="ps")
            for kc in range(2):
                for dt in range(4):
                    nc.tensor.matmul(
                        e2ps[0:1, kc * 512:(kc + 1) * 512],
                        negones[:],
                        cbsq[:, dt, kc * 512:(kc + 1) * 512],
                        start=(dt == 0), stop=(dt == 3),
                    )
            e2row = cp.tile([1, K], DT, tag="e2row")
            nc.scalar.activation(e2row[:], e2ps[0:1, :], AF.Copy)

            # ---- per-(b-tile) accumulators, written one column per tile ----
            Z_sb = mp.tile([128, BT], DT, tag="Z")
            SPS_sb = mp.tile([128, BT], DT, tag="SPS")
            ZQ_sb = mp.tile([128, BT], DT, tag="ZQ")
            RQ_sb = mp.tile([128, BT], DT, tag="RQ")
            PM_sb = mp.tile([128, BT], DT, tag="PM")
            idx_sb = mp.tile([128, BT * 8], mybir.dt.uint32, tag="idx")

            for c in range(NCHUNK):
                bs = c * 512
                inT_t = sp.tile([128, 4, 512], DT, tag="inT")
                nc.sync.dma_start(
                    inT_t[:],
                    inT_h.rearrange("(t p) b -> p t b", p=128)[:, :, bs:bs + 512],
                )
                g_t = sp.tile([128, 4, K], DT, tag="g")
                nc.sync.dma_start(
                    g_t[:],
                    g_h[bs:bs + 512, :].rearrange("(t p) k -> p t k", p=128),
                )

                # MM-A: xT[j, b] = 2 * (W_n^T @ inputs^T)[j, b] + 2*b_proj[j]
                xT_t = sp.tile([128, 4, 512], DT, tag="xT")
                for jt in range(4):
                    xps = pp.tile([128, K], DT, tag="ps")
                    for dt in range(4):
                        nc.tensor.matmul(
                            xps[:, :512],
                            w_sb[:, dt, jt * 128:(jt + 1) * 128],
                            inT_t[:, dt, :],
                            start=(dt == 0), stop=(dt == 3),
                        )
                    nc.scalar.activation(xT_t[:, jt, :], xps[:, :512],
                                         AF.Identity, scale=2.0,
                                         bias=bias_sb[:, jt:jt + 1])

                for bt in range(4):
                    col = c * 4 + bt
                    # MM-B: s[b, k] = 2*x.e - e2  (e2 via extra K=1 row)
                    # dt-outer so each stationary xT block serves both kc
                    # chunks (halves LDWEIGHTS count).
                    s_ps = pp.tile([128, K], DT, tag="ps")
                    for dt in range(4):
                        for kc in range(2):
                            ks = kc * 512
                            nc.tensor.matmul(
                                s_ps[:, ks:ks + 512],
                                xT_t[:, dt, bt * 128:(bt + 1) * 128],
                                cbT_sb[:, dt, ks:ks + 512],
                                start=(dt == 0), stop=False,
                            )
                    for kc in range(2):
                        ks = kc * 512
                        nc.tensor.matmul(
                            s_ps[:, ks:ks + 512],
                            ones_row[:],
                            e2row[:, ks:ks + 512],
                            start=False, stop=True,
                        )

                    # u = s + g  (for the gumbel-softmax sample)
                    u_t = sp.tile([128, K], DT, tag="u")
                    nc.vector.scalar_tensor_tensor(
                        u_t[:], s_ps[:], 1.0, g_t[:, bt, :],
                        op0=OP.mult, op1=OP.add)
                    # p = exp(s), Z = sum_k p
                    p_t = sp.tile([128, K], DT, tag="p")
                    nc.scalar.activation(p_t[:], s_ps[:], AF.Exp,
                                         accum_out=Z_sb[:, col:col + 1])
                    # sum_k p*s
                    ps_t = sp.tile([128, K], DT, tag="psjunk")
                    nc.vector.scalar_tensor_tensor(
                        ps_t[:], p_t[:], 1.0, s_ps[:],
                        op0=OP.mult, op1=OP.mult,
                        accum_out=SPS_sb[:, col:col + 1])
                    # q = exp(2u), Zq = sum_k q
                    q_t = sp.tile([128, K], FR, tag="q")
                    nc.scalar.activation(q_t[:], u_t[:], AF.Exp, scale=2.0,
                                         accum_out=ZQ_sb[:, col:col + 1])
                    # argmax_k s == argmax_k p (first occurrence)
                    nc.vector.tensor_reduce(PM_sb[:, col:col + 1], p_t[:],
                                            axis=mybir.AxisListType.X,
                                            op=OP.max)
                    nc.vector.max_index(
                        idx_sb[:, col * 8:(col + 1) * 8],
                        PM_sb[:, col:col + 1].broadcast_to([128, 8]),
                        p_t[:])

                    # qT via PE transpose, then quantized = qT.T @ cb / Zq
                    qT_ps = pp.tile([128, K], FR, tag="ps")
                    for kt in range(8):
                        nc.tensor.transpose(
                            qT_ps[:, kt * 128:(kt + 1) * 128],
                            q_t[:, kt * 128:(kt + 1) * 128],
                            id_sb[:])
                    qT_t = sp.tile([128, K], FR, tag="qT")
                    nc.scalar.activation(qT_t[:], qT_ps[:], AF.Copy)
                    nc.vector.reciprocal(RQ_sb[:, col:col + 1],
                                         ZQ_sb[:, col:col + 1])
                    qu_ps = pp.tile([128, K], DT, tag="ps")
                    for kt in range(8):
                        nc.tensor.matmul(
                            qu_ps[:, :512],
                            qT_t[:, kt * 128:(kt + 1) * 128],
                            cb_sb[:, kt, :],
                            start=(kt == 0), stop=(kt == 7),
                        )
                    qo_t = sp.tile([128, 512], DT, tag="qo")
                    nc.scalar.activation(qo_t[:], qu_ps[:, :512], AF.Copy,
                                         scale=RQ_sb[:, col:col + 1])
                    nc.sync.dma_start(quant_h[col * 128:(col + 1) * 128, :],
                                      qo_t[:])

            # ---- loss tail: KLsum = SPS/Z - ln Z + ln K, per (b-tile col) ----
            lz = mp.tile([128, BT], DT, tag="lz")
            nc.scalar.activation(lz[:], Z_sb[:], AF.Ln)
            rz = mp.tile([128, BT], DT, tag="rz")
            nc.vector.reciprocal(rz[:], Z_sb[:])
            t1 = mp.tile([128, BT], DT, tag="t1")
            nc.vector.tensor_mul(t1[:], SPS_sb[:], rz[:])
            kl_t = mp.tile([128, BT], DT, tag="klt")
            nc.vector.scalar_tensor_tensor(kl_t[:], t1[:], LOGK, lz[:],
                                           op0=OP.add, op1=OP.subtract)
            nc.sync.dma_start(kl_h[:], kl_t[:])
            nc.sync.dma_start(idx_h[:], idx_sb[:])

    nc.compile()
    _cache["nc"] = nc
    return nc


def _in_maps(inputs, W_proj, b_proj, codebooks):
    g = _gumbel()
    inT = np.ascontiguousarray(inputs.T)                      # [D, B]
    ident = np.eye(128, dtype=np.float32)
    maps = []
    for n in range(NCORES):
        maps.append({
            "inT": inT,
            "w": np.ascontiguousarray(W_proj[:, n * D:(n + 1) * D]),
            "bias": np.ascontiguousarray(
                2.0 * b_proj[n * D:(n + 1) * D].reshape(4, 128).T),
            "cbT": np.ascontiguousarray(codebooks[n].T),
            "cb": np.ascontiguousarray(codebooks[n]),
            "g": np.ascontiguousarray(g[:, n, :]),
            "ident": ident,
        })
    return maps


def kernel(inputs, var, W_proj, b_proj, codebooks):
    from concourse.bass_utils import run_bass_kernel_spmd

    inputs = np.asarray(inputs, dtype=np.float32)
    W_proj = np.asarray(W_proj, dtype=np.float32)
    b_proj = np.asarray(b_proj, dtype=np.float32)
    codebooks = np.asarray(codebooks, dtype=np.float32)

    nc = _build()
    maps = _in_maps(inputs, W_proj, b_proj, codebooks)
    res = run_bass_kernel_spmd(nc, maps, core_ids=list(range(NCORES)))

    quant = np.empty((B, N, D), dtype=np.float32)
    idx = np.empty((B, N), dtype=np.int32)
    kl_total = 0.0
    for n in range(NCORES):
        out = res.results[n]
        quant[:, n, :] = out["quant"]
        # idx buffer: [128 partitions, 32 cols * 8 slots]; slot 0 = argmax.
        # b = col*128 + partition.
        idxbuf = out["idx"].reshape(128, BT, 8)[:, :, 0]      # [p, col]
        idx[:, n] = idxbuf.T.reshape(B).astype(np.int32)
        klbuf = out["kl"]                                     # [p, col]
        kl_total += float(np.sum(klbuf.T, dtype=np.float64))
    loss = np.float32(kl_total / (B * N))
    return quant, loss, idx


# revision 14
# speedup vs baseline: 1.3577x; 1.3577x over previous
"""Trainium2 Bass kernel for the DVQ (decomposed vector-quantization) module.

Problem shapes (hardcoded): B=4096, D=512, K=1024, N=8, TEMP=0.5.

Sharding: one decomposition slice n per NeuronCore (8 cores = N=8).
  - replicated : inputs (transposed on host to [D, B])
  - sharded    : W_proj[:, n*D:(n+1)*D], b_proj[n*D:(n+1)*D], codebooks[n],
                 gumbel noise g[:, n, :]
  - gathered   : quantized (stack over n), encoding_indices (stack), loss
                 (mean of per-(b,n) KL sums; the "all-reduce" of the hint).

Math notes: with s[b,k] = 2*x.e - |e|^2, both argmin(distance) and
log_softmax(-distance) are invariant to the per-(b,n) shift |x|^2, so the
kernel never computes x2.  logits = s - log(sum_k exp(s));
KLsum(b) = sum_k p*s / Z - log Z + log K with p = exp(s), Z = sum p.
Gumbel-softmax: sample_probs = exp(2*(s+g)) / Zq (temp 0.5), and
quantized = (1/Zq) * sum_k exp(2(s+g))[k] * cb[k, :].
Value ranges keep exp() in f32 range without max-subtraction.
"""

import numpy as np

B, D, K, N = 4096, 512, 1024, 8
NCORES = 8
BT = B // 128          # 32 b-tiles of 128 rows
NCHUNK = 8             # b processed in 8 chunks of 512 rows
LOGK = float(np.log(K))

_cache = {}


def _gumbel() -> np.ndarray:
    """The reference's gumbel noise: jax.random.gumbel(key(1), (B, N, K))."""
    if "g" not in _cache:
        import jax

        with jax.default_device(jax.devices("cpu")[0]):
            g = jax.random.gumbel(
                jax.random.key(1), (B, N, K), dtype=np.float32
            )
            _cache["g"] = np.asarray(g)
    return _cache["g"]


def _build():
    """Build + compile the per-core Bass program (identical on all cores)."""
    if "nc" in _cache:
        return _cache["nc"]

    import concourse.bacc as bacc
    import concourse.mybir as mybir
    import concourse.tile as tile

    DT = mybir.dt.float32
    FR = mybir.dt.float32r
    AF = mybir.ActivationFunctionType
    OP = mybir.AluOpType

    nc = bacc.Bacc("TRN2", target_bir_lowering=False, debug=False,
                   num_devices=NCORES)

    inT_h = nc.dram_tensor("inT", [D, B], DT, kind="ExternalInput").ap()
    w_h = nc.dram_tensor("w", [D, D], DT, kind="ExternalInput").ap()
    bias_h = nc.dram_tensor("bias", [128, 4], DT, kind="ExternalInput").ap()
    cbT_h = nc.dram_tensor("cbT", [D, K], DT, kind="ExternalInput").ap()
    cb_h = nc.dram_tensor("cb", [K, D], FR, kind="ExternalInput").ap()
    g_h = nc.dram_tensor("g", [B, K], DT, kind="ExternalInput").ap()
    id_h = nc.dram_tensor("ident", [128, 128], FR, kind="ExternalInput").ap()

    quant_h = nc.dram_tensor("quant", [B, D], DT, kind="ExternalOutput").ap()
    idx_h = nc.dram_tensor("idx", [128, BT * 8], mybir.dt.uint32,
                           kind="ExternalOutput").ap()
    kl_h = nc.dram_tensor("kl", [128, BT], DT, kind="ExternalOutput").ap()

    with tile.TileContext(nc) as tc:
        with (
            tc.tile_pool(name="const", bufs=1) as cp,
            tc.tile_pool(name="stream", bufs=2) as sp,
            tc.tile_pool(name="stream3", bufs=3) as sp3,
            tc.tile_pool(name="small", bufs=1) as mp,
            tc.tile_pool(name="psum_s", bufs=2, space="PSUM") as pp_s,
            tc.tile_pool(name="psum_t", bufs=1, space="PSUM") as pp_t,
            tc.tile_pool(name="psum_q", bufs=2, space="PSUM") as pp_q,
        ):
            # ---- resident constants ----
            w_sb = cp.tile([128, 4, D], DT, tag="w")
            nc.sync.dma_start(w_sb[:], w_h.rearrange("(t p) j -> p t j", p=128))
            cbT_sb = cp.tile([128, 4, K], DT, tag="cbT")
            nc.sync.dma_start(cbT_sb[:], cbT_h.rearrange("(t p) k -> p t k", p=128))
            cb_sb = cp.tile([128, 8, D], FR, tag="cb")
            nc.sync.dma_start(cb_sb[:], cb_h.rearrange("(t p) d -> p t d", p=128))
            bias_sb = cp.tile([128, 4], DT, tag="bias")
            nc.sync.dma_start(bias_sb[:], bias_h[:])
            id_sb = cp.tile([128, 128], FR, tag="ident")
            nc.sync.dma_start(id_sb[:], id_h[:])

            ones_row = cp.tile([1, 128], DT, tag="ones_row")
            nc.gpsimd.memset(ones_row[:], 1.0)
            negones = cp.tile([128, 1], DT, tag="negones")
            nc.gpsimd.memset(negones[:], -1.0)

            # ---- e2row[0, k] = -sum_d cb[k, d]^2, in free-dim layout ----
            cbsq = cp.tile([128, 4, K], DT, tag="cbsq")
            for dt in range(4):
                nc.scalar.activation(cbsq[:, dt, :], cbT_sb[:, dt, :], AF.Square)
            e2ps = pp_s.tile([128, K], DT, tag="s")
            for kc in range(2):
                for dt in range(4):
                    nc.tensor.matmul(
                        e2ps[0:1, kc * 512:(kc + 1) * 512],
                        negones[:],
                        cbsq[:, dt, kc * 512:(kc + 1) * 512],
                        start=(dt == 0), stop=(dt == 3),
                    )
            e2row = cp.tile([1, K], DT, tag="e2row")
            nc.scalar.activation(e2row[:], e2ps[0:1, :], AF.Copy)

            # ---- per-(b-tile) accumulators, written one column per tile ----
            Z_sb = mp.tile([128, BT], DT, tag="Z")
            SPS_sb = mp.tile([128, BT], DT, tag="SPS")
            ZQ_sb = mp.tile([128, BT], DT, tag="ZQ")
            RQ_sb = mp.tile([128, BT], DT, tag="RQ")
            PM_sb = mp.tile([128, BT], DT, tag="PM")
            idx_sb = mp.tile([128, BT * 8], mybir.dt.uint32, tag="idx")

            def _emit_sample_mm(job):
                """qT via PE transpose, quantized = (qT.T @ cb) / Zq, DMA out."""
                q_prev, pcol = job
                qT_ps = pp_t.tile([128, K], FR, tag="qT")
                for kt in range(8):
                    nc.tensor.transpose(
                        qT_ps[:, kt * 128:(kt + 1) * 128],
                        q_prev[:, kt * 128:(kt + 1) * 128],
                        id_sb[:])
                qT_t = sp.tile([128, K], FR, tag="qTsb")
                nc.scalar.activation(qT_t[:], qT_ps[:], AF.Copy)
                nc.vector.reciprocal(RQ_sb[:, pcol:pcol + 1],
                                     ZQ_sb[:, pcol:pcol + 1])
                qu_ps = pp_q.tile([128, 512], DT, tag="qu")
                for kt in range(8):
                    nc.tensor.matmul(
                        qu_ps[:],
                        qT_t[:, kt * 128:(kt + 1) * 128],
                        cb_sb[:, kt, :],
                        start=(kt == 0), stop=(kt == 7),
                    )
                qo_t = sp.tile([128, 512], DT, tag="qo")
                nc.scalar.activation(qo_t[:], qu_ps[:], AF.Copy,
                                     scale=RQ_sb[:, pcol:pcol + 1])
                nc.sync.dma_start(quant_h[pcol * 128:(pcol + 1) * 128, :],
                                  qo_t[:])

            pending = None
            for c in range(NCHUNK):
                bs = c * 512
                inT_t = sp.tile([128, 4, 512], DT, tag="inT")
                nc.sync.dma_start(
                    inT_t[:],
                    inT_h.rearrange("(t p) b -> p t b", p=128)[:, :, bs:bs + 512],
                )
                g_t = sp.tile([128, 4, K], DT, tag="g")
                nc.sync.dma_start(
                    g_t[:],
                    g_h[bs:bs + 512, :].rearrange("(t p) k -> p t k", p=128),
                )

                # MM-A: xT[j, b] = 2 * (W_n^T @ inputs^T)[j, b] + 2*b_proj[j]
                xT_t = sp.tile([128, 4, 512], DT, tag="xT")
                for jt in range(4):
                    xps = pp_q.tile([128, 512], DT, tag="qu")
                    for dt in range(4):
                        nc.tensor.matmul(
                            xps[:],
                            w_sb[:, dt, jt * 128:(jt + 1) * 128],
                            inT_t[:, dt, :],
                            start=(dt == 0), stop=(dt == 3),
                        )
                    nc.scalar.activation(xT_t[:, jt, :], xps[:],
                                         AF.Identity, scale=2.0,
                                         bias=bias_sb[:, jt:jt + 1])

                for bt in range(4):
                    col = c * 4 + bt
                    # MM-B: s[b, k] = 2*x.e - e2  (e2 via extra K=1 row)
                    # dt-outer so each stationary xT block serves both kc
                    # chunks (halves LDWEIGHTS count).
                    s_ps = pp_s.tile([128, K], DT, tag="s")
                    for dt in range(4):
                        for kc in range(2):
                            ks = kc * 512
                            nc.tensor.matmul(
                                s_ps[:, ks:ks + 512],
                                xT_t[:, dt, bt * 128:(bt + 1) * 128],
                                cbT_sb[:, dt, ks:ks + 512],
                                start=(dt == 0), stop=False,
                            )
                    for kc in range(2):
                        ks = kc * 512
                        nc.tensor.matmul(
                            s_ps[:, ks:ks + 512],
                            ones_row[:],
                            e2row[:, ks:ks + 512],
                            start=False, stop=True,
                        )

                    # u = s + g  (for the gumbel-softmax sample)
                    u_t = sp.tile([128, K], DT, tag="u")
                    nc.vector.scalar_tensor_tensor(
                        u_t[:], s_ps[:], 1.0, g_t[:, bt, :],
                        op0=OP.mult, op1=OP.add)
                    # p = exp(s), Z = sum_k p
                    p_t = sp.tile([128, K], DT, tag="p")
                    nc.scalar.activation(p_t[:], s_ps[:], AF.Exp,
                                         accum_out=Z_sb[:, col:col + 1])
                    # sum_k p*s
                    ps_t = sp.tile([128, K], DT, tag="psjunk")
                    nc.vector.scalar_tensor_tensor(
                        ps_t[:], p_t[:], 1.0, s_ps[:],
                        op0=OP.mult, op1=OP.mult,
                        accum_out=SPS_sb[:, col:col + 1])
                    # q = exp(2u), Zq = sum_k q
                    q_t = sp3.tile([128, K], FR, tag="q")
                    nc.scalar.activation(q_t[:], u_t[:], AF.Exp, scale=2.0,
                                         accum_out=ZQ_sb[:, col:col + 1])
                    # argmax_k s == argmax_k p (first occurrence)
                    nc.vector.tensor_reduce(PM_sb[:, col:col + 1], p_t[:],
                                            axis=mybir.AxisListType.X,
                                            op=OP.max)
                    nc.vector.max_index(
                        idx_sb[:, col * 8:(col + 1) * 8],
                        PM_sb[:, col:col + 1].broadcast_to([128, 8]),
                        p_t[:])

                    # Software pipeline: the transpose+MM-C stage for the
                    # PREVIOUS tile is emitted after this tile's MM-B, so the
                    # PE never sits idle waiting for ACT's exp(q) and stays
                    # out of HAM re-throttle.
                    if pending is not None:
                        _emit_sample_mm(pending)
                    pending = (q_t, col)

            if pending is not None:
                _emit_sample_mm(pending)

            # ---- loss tail: KLsum = SPS/Z - ln Z + ln K, per (b-tile col) ----
            lz = mp.tile([128, BT], DT, tag="lz")
            nc.scalar.activation(lz[:], Z_sb[:], AF.Ln)
            rz = mp.tile([128, BT], DT, tag="rz")
            nc.vector.reciprocal(rz[:], Z_sb[:])
            t1 = mp.tile([128, BT], DT, tag="t1")
            nc.vector.tensor_mul(t1[:], SPS_sb[:], rz[:])
            kl_t = mp.tile([128, BT], DT, tag="klt")
            nc.vector.scalar_tensor_tensor(kl_t[:], t1[:], LOGK, lz[:],
                                           op0=OP.add, op1=OP.subtract)
            nc.sync.dma_start(kl_h[:], kl_t[:])
            nc.sync.dma_start(idx_h[:], idx_sb[:])

    nc.compile()
    _cache["nc"] = nc
    return nc


def _in_maps(inputs, W_proj, b_proj, codebooks):
    g = _gumbel()
    inT = np.ascontiguousarray(inputs.T)                      # [D, B]
    ident = np.eye(128, dtype=np.float32)
    maps = []
    for n in range(NCORES):
        maps.append({
            "inT": inT,
            "w": np.ascontiguousarray(W_proj[:, n * D:(n + 1) * D]),
            "bias": np.ascontiguousarray(
                2.0 * b_proj[n * D:(n + 1) * D].reshape(4, 128).T),
            "cbT": np.ascontiguousarray(codebooks[n].T),
            "cb": np.ascontiguousarray(codebooks[n]),
            "g": np.ascontiguousarray(g[:, n, :]),
            "ident": ident,
        })
    return maps


def kernel(inputs, var, W_proj, b_proj, codebooks):
    from concourse.bass_utils import run_bass_kernel_spmd

    inputs = np.asarray(inputs, dtype=np.float32)
    W_proj = np.asarray(W_proj, dtype=np.float32)
    b_proj = np.asarray(b_proj, dtype=np.float32)
    codebooks = np.asarray(codebooks, dtype=np.float32)

    nc = _build()
    maps = _in_maps(inputs, W_proj, b_proj, codebooks)
    res = run_bass_kernel_spmd(nc, maps, core_ids=list(range(NCORES)))

    quant = np.empty((B, N, D), dtype=np.float32)
    idx = np.empty((B, N), dtype=np.int32)
    kl_total = 0.0
    for n in range(NCORES):
        out = res.results[n]
        quant[:, n, :] = out["quant"]
        # idx buffer: [128 partitions, 32 cols * 8 slots]; slot 0 = argmax.
        # b = col*128 + partition.
        idxbuf = out["idx"].reshape(128, BT, 8)[:, :, 0]      # [p, col]
        idx[:, n] = idxbuf.T.reshape(B).astype(np.int32)
        klbuf = out["kl"]                                     # [p, col]
        kl_total += float(np.sum(klbuf.T, dtype=np.float64))
    loss = np.float32(kl_total / (B * N))
    return quant, loss, idx
